# revision 10
# baseline (speedup 1.0000x reference)
"""BiaffineAttn Trainium2 kernel.

Math (per batch b):
    t    = x2 @ U                      [S, D]
    attn = t @ x1^T + (x1 @ bias)[None, :]
    p    = softmax(attn, axis=-1)
    out  = relu((p @ x1) @ fc_w^T + fc_b)    [S, F]

Sharding: data-parallel over batch B=8, one batch per NeuronCore.

Per-core pipeline (all matmuls in fp32r = fp22-truncated fp32, which streams
at 1 cycle/row on the PE vs 4 for true fp32; N=512 moving chunks):
  The whole attention block is computed in TRANSPOSED orientation so that the
  softmax key dimension (t') lands on SBUF partitions:
    tT      = (x2 @ U)^T          stationary U[d,e] chunks, moving x2T[d,s]
    scoresT = attn^T [t', s]      stationary x1T[e,t'] 128x128 tiles, moving tT
    pT      = exp(scoresT - rowmax_bcast + kb)   (exact per-row max; kb is the
                                   per-key additive bias folded into exp's
                                   per-partition bias operand)
    oT      = (p @ x1)^T          stationary x1[t',e] chunks, moving pT
    outT    = relu((oT^T @ fcwT)^T * recip + fcb) stationary fcwT[e,f], moving oT
  rowmax: running elementwise max over the 16 t'-tiles of scoresT, then a
  128-partition reduce via 4 PE transposes, reassembled into a broadcast tile
  with K=1 ones-matmuls.  rowsum: ones-column matmuls accumulating over t'.

Host side: transposes x1/x2/fc_w per-core (layout prep for DMA-efficient
loads; fp32 DMA transpose does not exist on TRN2) and transposes the [F,S]
per-core output back to [S,F] when gathering.
"""

import os
import sys
from contextlib import ExitStack

import numpy as np

for _p in ("/opt/trn_rl_repo", os.path.expanduser("~/.axon_site/_ro/trn_rl_repo")):
    if os.path.isdir(_p) and _p not in sys.path:
        sys.path.insert(0, _p)

import concourse.bass as bass
import concourse.mybir as mybir
import concourse.tile as tile
from concourse import bacc

B = 8
S = 2048          # sequence length (both s and t')
D = 1024          # d_model
F = 512           # fc output dim
P = 128
SB = 512          # s superblock (moving free dim of every matmul)
NSB = S // SB     # 4
DC = D // P       # 8 contraction chunks of d / e
TC = S // P       # 16 t' tiles
FP32 = mybir.dt.float32
FP32R = mybir.dt.float32r
AF = mybir.ActivationFunctionType
ALU = mybir.AluOpType
AX = mybir.AxisListType


def build_nc():
    nc = bacc.Bacc(
        "TRN2",
        target_bir_lowering=False,
        debug=False,
        enable_asserts=False,
    )

    x1_d = nc.dram_tensor("x1", [S, D], FP32R, kind="ExternalInput")
    x1t_d = nc.dram_tensor("x1t", [D, S], FP32R, kind="ExternalInput")
    x2t_d = nc.dram_tensor("x2t", [D, S], FP32R, kind="ExternalInput")
    u_d = nc.dram_tensor("u", [D, D], FP32R, kind="ExternalInput")
    fcwt_d = nc.dram_tensor("fcwt", [D, F], FP32R, kind="ExternalInput")
    bias_d = nc.dram_tensor("bias", [1, D], FP32R, kind="ExternalInput")
    fcb_d = nc.dram_tensor("fcb", [F, 1], FP32, kind="ExternalInput")
    outt_d = nc.dram_tensor("outt", [F, S], FP32, kind="ExternalOutput")

    with tile.TileContext(nc) as tc, ExitStack() as ctx:
        # ---------- persistent pools ----------
        p_x1 = ctx.enter_context(tc.tile_pool(name="x1res", bufs=TC))
        p_u = ctx.enter_context(tc.tile_pool(name="ures", bufs=DC))
        p_kb = ctx.enter_context(tc.tile_pool(name="kbcols", bufs=TC))
        p_fcb = ctx.enter_context(tc.tile_pool(name="fcbcols", bufs=F // P))
        p_ones = ctx.enter_context(tc.tile_pool(name="ones", bufs=1))
        p_psum = ctx.enter_context(
            tc.tile_pool(name="psum", bufs=8, space="PSUM")
        )

        # resident x1 (natural layout, row tiles)
        x1_tiles = []
        for i in range(TC):
            x1_t = p_x1.tile([P, D], FP32R, name=f"x1r{i}", tag="x1r")
            nc.sync.dma_start(x1_t[:], x1_d[i * P : (i + 1) * P, :])
            x1_tiles.append(x1_t)
        # resident U (d-chunk rows; stationary slices [:, e_tile])
        u_tiles = []
        for i in range(DC):
            u_t = p_u.tile([P, D], FP32R, name=f"ur{i}", tag="ur")
            nc.sync.dma_start(u_t[:], u_d[i * P : (i + 1) * P, :])
            u_tiles.append(u_t)
        # fc_b as per-partition columns
        fcb_cols = []
        for i in range(F // P):
            c_t = p_fcb.tile([P, 1], FP32, name=f"fcb{i}", tag="fcb")
            nc.sync.dma_start(c_t[:], fcb_d[i * P : (i + 1) * P, :])
            fcb_cols.append(c_t)

        identity = p_ones.tile([P, P], FP32, name="ident", tag="ident", bufs=1)
        nc.gpsimd.memset(identity[:], 0.0)
        # ones tiles must be written as fp32r (fp32r-matmul operand rule);
        # memset can't emit fp32r, so use ACT: out = Copy(in*0 + 1).
        ones_row = p_ones.tile([1, P], FP32R, name="ones_row", tag="ones_row")
        nc.scalar.activation(ones_row[:], identity[0:1, :], AF.Identity, bias=1.0, scale=0.0)
        ones_col = p_ones.tile([P, 1], FP32R, name="ones_col", tag="ones_col")
        nc.scalar.activation(ones_col[:], identity[:, 0:1], AF.Identity, bias=1.0, scale=0.0)
        nc.gpsimd.affine_select(
            out=identity[:],
            in_=identity[:],
            compare_op=ALU.not_equal,
            fill=1.0,
            base=0,
            pattern=[[-1, P]],
            channel_multiplier=1,
        )

        # ---------- phase 0: kb columns (kb = x1 @ bias, per t' partition) ----
        kb_cols = []
        with tc.tile_pool(name="kbtmp", bufs=1) as p_kbt:
            bias_row = p_kbt.tile([1, D], FP32R, name="bias_row", tag="brow")
            nc.sync.dma_start(bias_row[:], bias_d[:, :])
            bias_bc = p_kbt.tile([P, D], FP32, name="bias_bc", tag="bbc")
            for h in range(D // SB):
                ps_b = p_psum.tile([P, SB], FP32, name=f"psb{h}", tag="ps")
                nc.tensor.matmul(
                    ps_b[:],
                    (ones_row[:]),
                    (bias_row[:, h * SB : (h + 1) * SB]),
                    start=True,
                    stop=True,
                )
                nc.vector.tensor_copy(bias_bc[:, h * SB : (h + 1) * SB], ps_b[:])
            dump = p_kbt.tile([P, D], FP32, name="kbdump", tag="kbdump")
            for i in range(TC):
                kb_c = p_kb.tile([P, 1], FP32, name=f"kb{i}", tag="kb")
                nc.vector.tensor_mul(
                    dump[:], x1_tiles[i][:].bitcast(FP32), bias_bc[:]
                )
                nc.vector.reduce_sum(kb_c[:], dump[:], axis=AX.X)
                kb_cols.append(kb_c)

        # ---------- streaming pools for the superblock loop ----------
        p_x2t = ctx.enter_context(tc.tile_pool(name="x2ts", bufs=8))
        p_x1t = ctx.enter_context(tc.tile_pool(name="x1ts", bufs=8))
        p_tt = ctx.enter_context(tc.tile_pool(name="tts", bufs=9))
        p_sc = ctx.enter_context(tc.tile_pool(name="scores", bufs=TC))
        p_ot = ctx.enter_context(tc.tile_pool(name="ots", bufs=8))
        p_aux = ctx.enter_context(tc.tile_pool(name="aux", bufs=1))
        p_row = ctx.enter_context(tc.tile_pool(name="rows", bufs=1))
        p_out = ctx.enter_context(tc.tile_pool(name="outs", bufs=2))
        p_fcw = ctx.enter_context(tc.tile_pool(name="fcws", bufs=6))

        for sb in range(NSB):
            s0 = sb * SB

            # ---- MM1: tT[:, sb] = (x2 @ U)^T superblock columns ----
            x2t_tiles = []
            for dc in range(DC):
                x2_t = p_x2t.tile([P, SB], FP32R, name=f"x2t_{sb}_{dc}", tag="x2t")
                nc.sync.dma_start(x2_t[:], x2t_d[dc * P : (dc + 1) * P, s0 : s0 + SB])
                x2t_tiles.append(x2_t)
            tt_tiles = []
            for eh in range(2):  # split e into halves: 4 concurrent psum banks
                ps_t = [
                    p_psum.tile([P, SB], FP32, name=f"pst{sb}_{eh}_{i}", tag="ps")
                    for i in range(4)
                ]
                for dc in range(DC):
                    for i in range(4):
                        et = eh * 4 + i
                        nc.tensor.matmul(
                            ps_t[i][:],
                            (u_tiles[dc][:, et * P : (et + 1) * P]),
                            (x2t_tiles[dc][:]),
                            start=(dc == 0),
                            stop=(dc == DC - 1),
                        )
                for i in range(4):
                    t_t = p_tt.tile([P, SB], FP32R, name=f"tt{sb}_{eh}_{i}", tag="tt")
                    nc.vector.tensor_copy(t_t[:], ps_t[i][:])
                    tt_tiles.append(t_t)

            # ---- MM2: scoresT tiles + running elementwise max ----
            sc_tiles = []
            maxacc = p_aux.tile([P, SB], FP32, name=f"maxacc{sb}", tag="maxacc")
            for ti in range(TC):
                ps_s = p_psum.tile([P, SB], FP32, name=f"pss{sb}_{ti}", tag="ps")
                for ec in range(DC):
                    x1t_t = p_x1t.tile(
                        [P, P], FP32R, name=f"x1t_{sb}_{ti}_{ec}", tag="x1t"
                    )
                    nc.sync.dma_start(
                        x1t_t[:],
                        x1t_d[ec * P : (ec + 1) * P, ti * P : (ti + 1) * P],
                    )
                    nc.tensor.matmul(
                        ps_s[:],
                        (x1t_t[:]),
                        (tt_tiles[ec][:]),
                        start=(ec == 0),
                        stop=(ec == DC - 1),
                    )
                s_t = p_sc.tile([P, SB], FP32R, name=f"sc{sb}_{ti}", tag="sc")
                nc.vector.tensor_copy(s_t[:], ps_s[:])
                if ti == 0:
                    nc.scalar.copy(maxacc[:], ps_s[:])
                else:
                    nc.vector.tensor_max(maxacc[:], maxacc[:], ps_s[:])
                sc_tiles.append(s_t)

            # ---- per-s (free dim) max: 4 PE transposes + free reduce ----
            mrow = p_row.tile([1, SB], FP32R, name=f"mrow{sb}", tag="mrow")
            for blk in range(SB // P):
                ps_tr = p_psum.tile([P, P], FP32, name=f"ptr{sb}_{blk}", tag="ps")
                nc.tensor.transpose(
                    ps_tr[:], maxacc[:, blk * P : (blk + 1) * P], identity[:]
                )
                mcol = p_row.tile([P, 1], FP32, name=f"mcol{sb}_{blk}", tag="mcol")
                nc.vector.reduce_max(mcol[:], ps_tr[:], axis=AX.X)
                ps_rr = p_psum.tile([1, P], FP32, name=f"prr{sb}_{blk}", tag="ps")
                nc.tensor.transpose(ps_rr[:], mcol[:], identity[:])
                nc.vector.tensor_copy(mrow[:, blk * P : (blk + 1) * P], ps_rr[:])
            ps_mb = p_psum.tile([P, SB], FP32, name=f"pmb{sb}", tag="ps")
            nc.tensor.matmul(ps_mb[:], (ones_row[:]), (mrow[:]), start=True, stop=True)
            maxb = p_aux.tile([P, SB], FP32, name=f"maxb{sb}", tag="maxb")
            nc.vector.tensor_copy(maxb[:], ps_mb[:])

            # ---- exp(scores - maxb + kb) in place ----
            for ti in range(TC):
                nc.vector.tensor_sub(sc_tiles[ti][:], sc_tiles[ti][:].bitcast(FP32), maxb[:])
                nc.scalar.activation(
                    sc_tiles[ti][:],
                    sc_tiles[ti][:].bitcast(FP32),
                    AF.Exp,
                    bias=kb_cols[ti][:],
                    scale=1.0,
                )

            # ---- row sums over t' (ones-column matmuls) + recip broadcast ----
            ps_sum = p_psum.tile([1, SB], FP32, name=f"psum{sb}", tag="ps")
            for ti in range(TC):
                nc.tensor.matmul(
                    ps_sum[:],
                    (ones_col[:]),
                    (sc_tiles[ti][:]),
                    start=(ti == 0),
                    stop=(ti == TC - 1),
                )
            srow = p_row.tile([1, SB], FP32, name=f"srow{sb}", tag="srow")
            nc.vector.tensor_copy(srow[:], ps_sum[:])
            rrow = p_row.tile([1, SB], FP32R, name=f"rrow{sb}", tag="rrow")
            with nc.allow_low_precision(reason="recip feeds fp32r matmul; fp22 ok"):
                nc.vector.reciprocal(rrow[:], srow[:])
            ps_rb = p_psum.tile([P, SB], FP32, name=f"prb{sb}", tag="ps")
            nc.tensor.matmul(ps_rb[:], (ones_row[:]), (rrow[:]), start=True, stop=True)
            recipb = p_aux.tile([P, SB], FP32, name=f"recipb{sb}", tag="recipb")
            nc.vector.tensor_copy(recipb[:], ps_rb[:])

            # ---- MM4: oT = (p~ @ x1)^T (unnormalized) ----
            ot_tiles = []
            for et in range(DC):
                ps_o = p_psum.tile([P, SB], FP32, name=f"pso{sb}_{et}", tag="ps")
                for ti in range(TC):
                    nc.tensor.matmul(
                        ps_o[:],
                        (x1_tiles[ti][:, et * P : (et + 1) * P]),
                        (sc_tiles[ti][:]),
                        start=(ti == 0),
                        stop=(ti == TC - 1),
                    )
                o_t = p_ot.tile([P, SB], FP32R, name=f"ot{sb}_{et}", tag="ot")
                nc.vector.tensor_copy(o_t[:], ps_o[:])
                ot_tiles.append(o_t)

            # ---- MM5 + normalize + bias + relu + store ----
            for ft in range(F // P):
                ps_f = p_psum.tile([P, SB], FP32, name=f"psf{sb}_{ft}", tag="ps")
                for ec in range(DC):
                    fcw_t = p_fcw.tile(
                        [P, P], FP32R, name=f"fcw{sb}_{ft}_{ec}", tag="fcw"
                    )
                    nc.sync.dma_start(
                        fcw_t[:],
                        fcwt_d[ec * P : (ec + 1) * P, ft * P : (ft + 1) * P],
                    )
                    nc.tensor.matmul(
                        ps_f[:],
                        (fcw_t[:]),
                        (ot_tiles[ec][:]),
                        start=(ec == 0),
                        stop=(ec == DC - 1),
                    )
                tmp = p_out.tile([P, SB], FP32, name=f"tmp{sb}_{ft}", tag="tmp")
                nc.vector.tensor_mul(tmp[:], ps_f[:], recipb[:])
                o_out = p_out.tile([P, SB], FP32, name=f"oo{sb}_{ft}", tag="oo")
                nc.scalar.activation(
                    o_out[:], tmp[:], AF.Relu, bias=fcb_cols[ft][:], scale=1.0
                )
                nc.sync.dma_start(
                    outt_d[ft * P : (ft + 1) * P, s0 : s0 + SB], o_out[:]
                )

    nc.compile()
    return nc


_NC_CACHE = None


def _get_nc():
    global _NC_CACHE
    if _NC_CACHE is None:
        _NC_CACHE = build_nc()
    return _NC_CACHE


def make_in_maps(x1, x2, U, bias, fc_w, fc_b):
    x1 = np.ascontiguousarray(np.asarray(x1, dtype=np.float32))
    x2 = np.ascontiguousarray(np.asarray(x2, dtype=np.float32))
    U = np.ascontiguousarray(np.asarray(U, dtype=np.float32))
    bias = np.asarray(bias, dtype=np.float32).reshape(1, D)
    fc_w = np.asarray(fc_w, dtype=np.float32)
    fc_b = np.asarray(fc_b, dtype=np.float32).reshape(F, 1)
    fcwt = np.ascontiguousarray(fc_w.T)
    in_maps = []
    for b in range(B):
        in_maps.append(
            {
                "x1": np.ascontiguousarray(x1[b]),
                "x1t": np.ascontiguousarray(x1[b].T),
                "x2t": np.ascontiguousarray(x2[b].T),
                "u": U,
                "fcwt": fcwt,
                "bias": bias,
                "fcb": fc_b,
            }
        )
    return in_maps


def kernel(x1, x2, U, bias, fc_w, fc_b):
    from concourse.bass_utils import run_bass_kernel_spmd

    nc = _get_nc()
    in_maps = make_in_maps(x1, x2, U, bias, fc_w, fc_b)
    res = run_bass_kernel_spmd(nc, in_maps, core_ids=list(range(B)))
    out = np.stack([np.ascontiguousarray(r["outt"].T) for r in res.results])
    return out.astype(np.float32)
